# revision 29
# baseline (speedup 1.0000x reference)
"""Causal self-attention Trainium2 kernel (8 NeuronCores, SPMD).

Problem (hardcoded): x [4, 2048, 2048] f32, W_qkv [6144, 2048], W_out [2048, 2048],
16 heads x 128 dim, causal softmax attention + output projection.

Sharding: core c = 2*b + g handles batch b (4) and head-group g (2 groups of 8
heads).  Each core computes its 8 heads' QKV projection, attention, and the
partial output projection against its slice of W_out columns; the host sums the
two partials per batch element.

v3 design: all matmuls in bf16 (double-pumped on TRN2 silicon, ~2 cols/cycle;
no >=256 moving-width rule, so causal tiles use exact widths).  Single per-head
pipeline: attention of head h (ACT exp / DVE softmax work) interleaves with the
QKV projection matmuls of head h+1 (and, for head 0, with its own later QKV
tiles), keeping the PE busy while scalar work drains.  Q^T/K^T/V stay in SBUF
per head.  Exps run as fused 896-1024-wide ACT instructions over paired S
tiles.  The softmax denominator is accumulated with bf16 DVE chain adds,
all-reduced+broadcast across partitions in one Pool-engine op, and inverted on
DVE; the normalize multiply reads PSUM once.
"""

import math
from collections import deque

import numpy as np

B = 4
T = 2048
C = 2048
H = 16          # total heads
HG = 8          # heads per core (tensor-parallel group)
D = 128         # head dim
P = 128         # partitions
NCS = C // P    # 16 contraction subtiles
NTC = T // P    # 16 T chunks of 128
NTB = T // 512  # 4 T blocks of 512
SCALE = 1.0 / math.sqrt(D)

_CACHED = None
VMODE = "bf16"


def _build(phases="abc", repeat=1, vmode=None):
    import concourse.mybir as mybir
    from concourse import bacc
    from concourse import bass_isa
    from concourse.tile import TileContext

    f32 = mybir.dt.float32
    f32r = mybir.dt.float32r
    bf16 = mybir.dt.bfloat16
    EXP = mybir.ActivationFunctionType.Exp
    MULT = mybir.AluOpType.mult
    ADD = mybir.AluOpType.add
    RADD = bass_isa.ReduceOp.add

    nc = bacc.Bacc("TRN2", target_bir_lowering=False)

    xt_d = nc.dram_tensor("xt", [NCS, P, T], bf16, kind="ExternalInput")
    wq_d = nc.dram_tensor("wq", [HG, P, NCS, D], bf16, kind="ExternalInput")
    wk_d = nc.dram_tensor("wk", [HG, P, NCS, D], bf16, kind="ExternalInput")
    wv_d = nc.dram_tensor("wv", [HG, P, NCS, D], bf16, kind="ExternalInput")
    wo_d = nc.dram_tensor("wo", [HG * D, C], bf16, kind="ExternalInput")
    tri_d = nc.dram_tensor("tri", [P, P], bf16, kind="ExternalInput")
    ones_d = nc.dram_tensor("ones", [P, 1], bf16, kind="ExternalInput")
    out_d = nc.dram_tensor("out", [T, C], f32, kind="ExternalOutput")

    with TileContext(nc) as tc:
        with tc.tile_pool(name="persist", bufs=1) as persist:
            tri_t = persist.tile([P, P], bf16, tag="tri")
            nc.sync.dma_start(tri_t, tri_d[:])
            ones_t = persist.tile([P, 1], bf16, tag="ones")
            nc.sync.dma_start(ones_t, ones_d[:])

            for _rep in range(repeat):
                with tc.tile_pool(name="ot", bufs=1) as otp, \
                     tc.tile_pool(name="cw", bufs=2) as cwp, \
                     tc.tile_pool(name="cstage", bufs=4) as cstage:
                  wo_r = wo_d.rearrange("(h p) o -> p h o", p=P)
                  with tc.tile_pool(name="xt", bufs=1) as xtp, \
                     tc.tile_pool(name="wp", bufs=6) as wp, \
                     tc.tile_pool(name="qkv", bufs=2) as qkvsb, \
                     tc.tile_pool(name="pt1024", bufs=8) as pt1024, \
                     tc.tile_pool(name="pt512", bufs=2) as pt512, \
                     tc.tile_pool(name="acc", bufs=2) as accp, \
                     tc.tile_pool(name="misc", bufs=2) as mscp, \
                     tc.tile_pool(name="qkvp", bufs=2, space="PSUM") as qkvp, \
                     tc.tile_pool(name="sp1024", bufs=2, space="PSUM") as sp1024, \
                     tc.tile_pool(name="sp512", bufs=1, space="PSUM") as sp512, \
                     tc.tile_pool(name="dnp", bufs=1, space="PSUM") as dnp:

                    xt = []

                    def load_w(h):
                        wq_t = wp.tile([P, NCS, D], bf16, tag="w")
                        nc.sync.dma_start(wq_t, wq_d[h])
                        wk_t = wp.tile([P, NCS, D], bf16, tag="w")
                        nc.sync.dma_start(wk_t, wk_d[h])
                        wv_t = wp.tile([P, NCS, D], bf16, tag="w")
                        nc.sync.dma_start(wv_t, wv_d[h])
                        return wq_t, wk_t, wv_t

                    def qkv_units(h, wts):
                        """12 closures in tb-group-major order
                        (q_tb, k_tb, v_tb) x 4; each issues one psum tile's
                        worth of QKV matmuls for head h plus staging copy."""
                        wq_t, wk_t, wv_t = wts
                        qt_t = qkvsb.tile([P, T], bf16, tag="qt")
                        kt_t = qkvsb.tile([P, T], bf16, tag="kt")
                        v_t = qkvsb.tile([P, T], bf16, tag="vt")
                        units = []
                        cpn = [0]

                        def stage_copy(out, in_):
                            # alternate DVE/ACT so neither engine becomes
                            # the per-head bottleneck (Pool can't read PSUM)
                            cp = (nc.vector.tensor_copy if cpn[0] % 2 == 0
                                  else nc.scalar.copy)
                            cpn[0] += 1
                            cp(out=out, in_=in_)

                        for tb in range(NTB):
                            for w_t, dst in ((wq_t, qt_t), (wk_t, kt_t)):
                                def u(w_t=w_t, dst=dst, tb=tb):
                                    ps = qkvp.tile([P, 512], f32, tag="qp")
                                    for cs in range(NCS):
                                        nc.tensor.matmul(
                                            ps, w_t[:, cs],
                                            xt[cs][:, tb * 512:(tb + 1) * 512],
                                            start=(cs == 0),
                                            stop=(cs == NCS - 1))
                                    stage_copy(
                                        out=dst[:, tb * 512:(tb + 1) * 512],
                                        in_=ps)
                                units.append(u)

                            def u(g=tb, wv_t=wv_t, v_t=v_t):
                                ps = qkvp.tile([P, 512], f32, tag="qp")
                                for j in range(4):
                                    tc_ = 4 * g + j
                                    for cs in range(NCS):
                                        nc.tensor.matmul(
                                            ps[:, j * P:(j + 1) * P],
                                            xt[cs][:, tc_ * P:(tc_ + 1) * P],
                                            wv_t[:, cs],
                                            start=(cs == 0),
                                            stop=(cs == NCS - 1))
                                stage_copy(
                                    out=v_t[:, g * 512:(g + 1) * 512],
                                    in_=ps)
                            units.append(u)
                        return units, (qt_t, kt_t, v_t)

                    def att(h, qkv, next_unit, ensure_group=None,
                            jb_done=None):
                        qt_t, kt_t, v_t = qkv
                        ot_h = otp.tile([P, T], bf16, tag=f"ot{h}")
                        for jb in range(NTB):
                            if ensure_group is not None:
                                ensure_group(jb)
                            qb = slice(jb * 512, (jb + 1) * 512)
                            state = {"acc": None}
                            avs = []
                            tiles = 0

                            def add_acc(src, q0, width):
                                if state["acc"] is None:
                                    a = accp.tile([P, 512], bf16, tag="acc")
                                    nc.vector.tensor_copy(out=a, in_=src)
                                    state["acc"] = a
                                else:
                                    a = state["acc"]
                                    nc.vector.tensor_tensor(
                                        a[:, q0:q0 + width],
                                        a[:, q0:q0 + width], src, ADD)

                            # off-diagonal k-chunk pairs, exp fused 1024-wide
                            for i in range(2 * jb):
                                ks0, ks1 = 2 * i, 2 * i + 1
                                ps = sp1024.tile([P, 1024], f32, tag="sp")
                                nc.tensor.matmul(
                                    ps[:, 0:512],
                                    kt_t[:, ks0 * P:(ks0 + 1) * P],
                                    qt_t[:, qb], start=True, stop=True)
                                nc.tensor.matmul(
                                    ps[:, 512:1024],
                                    kt_t[:, ks1 * P:(ks1 + 1) * P],
                                    qt_t[:, qb], start=True, stop=True)
                                pt = pt1024.tile([P, 1024], bf16, tag="pt")
                                nc.scalar.activation(pt, ps, EXP, scale=SCALE)
                                add_acc(pt[:, 0:512], 0, 512)
                                add_acc(pt[:, 512:1024], 0, 512)
                                avs.append((ks0, pt[:, 0:512], 0))
                                avs.append((ks1, pt[:, 512:1024], 0))
                                tiles += 1
                                if tiles % 2 == 0:
                                    next_unit()

                            # diagonal chunks m=0 (w512) + m=1 (w384), fused
                            ka0, ka1 = 4 * jb, 4 * jb + 1
                            psA = sp1024.tile([P, 1024], f32, tag="sp")
                            nc.tensor.matmul(
                                psA[:, 0:512],
                                kt_t[:, ka0 * P:(ka0 + 1) * P],
                                qt_t[:, qb], start=True, stop=True)
                            nc.tensor.matmul(
                                psA[:, 512:896],
                                kt_t[:, ka1 * P:(ka1 + 1) * P],
                                qt_t[:, jb * 512 + 128:(jb + 1) * 512],
                                start=True, stop=True)
                            ptA = pt1024.tile([P, 1024], bf16, tag="pt")
                            nc.scalar.activation(
                                ptA[:, 0:896], psA[:, 0:896], EXP,
                                scale=SCALE)
                            nc.vector.tensor_tensor(
                                ptA[:, 0:P], ptA[:, 0:P], tri_t, MULT)
                            nc.vector.tensor_tensor(
                                ptA[:, 512:512 + P], ptA[:, 512:512 + P],
                                tri_t, MULT)
                            add_acc(ptA[:, 0:512], 0, 512)
                            add_acc(ptA[:, 512:896], 128, 384)
                            avs.append((ka0, ptA[:, 0:512], 0))
                            avs.append((ka1, ptA[:, 512:896], 128))
                            tiles += 1
                            if tiles % 2 == 0:
                                next_unit()

                            # diagonal chunks m=2 (w256) + m=3 (w128), fused
                            kb0, kb1 = 4 * jb + 2, 4 * jb + 3
                            psB = sp512.tile([P, 512], f32, tag="spb")
                            nc.tensor.matmul(
                                psB[:, 0:256],
                                kt_t[:, kb0 * P:(kb0 + 1) * P],
                                qt_t[:, jb * 512 + 256:(jb + 1) * 512],
                                start=True, stop=True)
                            nc.tensor.matmul(
                                psB[:, 256:384],
                                kt_t[:, kb1 * P:(kb1 + 1) * P],
                                qt_t[:, jb * 512 + 384:(jb + 1) * 512],
                                start=True, stop=True)
                            ptB = pt512.tile([P, 512], bf16, tag="ptb")
                            nc.scalar.activation(
                                ptB[:, 0:384], psB[:, 0:384], EXP,
                                scale=SCALE)
                            nc.vector.tensor_tensor(
                                ptB[:, 0:P], ptB[:, 0:P], tri_t, MULT)
                            nc.vector.tensor_tensor(
                                ptB[:, 256:384], ptB[:, 256:384], tri_t,
                                MULT)
                            add_acc(ptB[:, 0:256], 256, 256)
                            add_acc(ptB[:, 256:384], 384, 128)
                            avs.append((kb0, ptB[:, 0:256], 256))
                            avs.append((kb1, ptB[:, 256:384], 384))
                            tiles += 1
                            if tiles % 2 == 0:
                                next_unit()

                            # denominator: ones-matmul reduce (PE), DVE
                            # reciprocal, Pool broadcast; recip+broadcast
                            # overlap the AV matmuls below.
                            dn = dnp.tile([1, 512], f32, tag="dn")
                            nc.tensor.matmul(dn, ones_t, state["acc"],
                                             start=True, stop=True)
                            rc = mscp.tile([1, 512], f32r, tag="rc")
                            with nc.allow_low_precision("softmax recip"):
                                nc.vector.reciprocal(rc, dn)
                            rb = mscp.tile([P, 512], f32r, tag="rb")
                            nc.gpsimd.partition_broadcast(rb, rc)

                            po = sp512.tile([P, 512], f32, tag="spb")
                            nav = len(avs)
                            for idx, (ks, src, q0) in enumerate(avs):
                                nc.tensor.matmul(
                                    po[:, q0:], v_t[:, ks * P:(ks + 1) * P],
                                    src,
                                    start=(idx == 0), stop=(idx == nav - 1))
                            nc.vector.tensor_tensor(
                                ot_h[:, qb], po, rb, MULT)
                            if jb_done is not None:
                                jb_done(jb, ot_h)
                        return ot_h

                    # weights for heads 0/1 load before the x bulk; x^T
                    # loads tb-sliced so the first QKV tiles start early.
                    wts0 = load_w(0)
                    wts1_pre = load_w(1)
                    for cs in range(NCS):
                        t_ = xtp.tile([P, T], bf16, tag=f"xt{cs}")
                        xt.append(t_)
                    for tb in range(NTB):
                        for cs in range(NCS):
                            nc.sync.dma_start(
                                xt[cs][:, tb * 512:(tb + 1) * 512],
                                xt_d[cs][:, tb * 512:(tb + 1) * 512])
                    units0, qkv0 = qkv_units(0, wts0)
                    own = deque(units0)
                    own_issued = [0]
                    for _ in range(3):
                        own.popleft()()
                        own_issued[0] += 1
                    wts_next = wts1_pre

                    pending = deque()

                    def next_unit():
                        if own:
                            own.popleft()()
                            own_issued[0] += 1
                        elif pending:
                            pending.popleft()()

                    def ensure_group(jb):
                        while own_issued[0] < 3 * (jb + 1) and own:
                            own.popleft()()
                            own_issued[0] += 1

                    ot = []

                    def make_c_unit(tch, wo_t0, ots):
                        # one ob=0 output-projection tile; fills the PE
                        # during head-7 attention (copy on the idle Pool)
                        def u():
                            ps = qkvp.tile([P, 512], f32, tag="qp")
                            for h2 in range(HG):
                                nc.tensor.matmul(
                                    ps,
                                    ots[h2][:, tch * P:(tch + 1) * P],
                                    wo_t0[:, h2],
                                    start=(h2 == 0), stop=(h2 == HG - 1))
                            st = cstage.tile([P, 512], f32, tag="cst")
                            cp = (nc.vector.tensor_copy if tch % 2 == 0
                                  else nc.scalar.copy)
                            cp(out=st, in_=ps)
                            nc.sync.dma_start(
                                out_d[tch * P:(tch + 1) * P, 0:512], st)
                        return u

                    def make_jbd(wo_t0):
                        def jbd(jb, ot_h):
                            ots = ot + [ot_h]
                            for tch in range(4 * jb, 4 * jb + 4):
                                pending.append(
                                    make_c_unit(tch, wo_t0, ots))
                        return jbd

                    wo_t0 = None
                    qkv_cur = qkv0
                    for h in range(HG):
                        if h + 1 < HG:
                            units, qkv_next = qkv_units(h + 1, wts_next)
                            pending.extend(units)
                        if h + 2 < HG:
                            wts_next = load_w(h + 2)
                        if h == 6:
                            wo_t0 = cwp.tile([P, HG, 512], bf16, tag="cw")
                            nc.sync.dma_start(wo_t0, wo_r[:, :, 0:512])

                        ot.append(att(h, qkv_cur, next_unit,
                                      ensure_group if h == 0 else None,
                                      make_jbd(wo_t0) if h == 7 else None))
                        while own:
                            own.popleft()()
                            own_issued[0] += 1
                        while pending:
                            pending.popleft()()
                        if h + 1 < HG:
                            qkv_cur = qkv_next

                  # ------------- output projection (ob 1..3) -------------
                  # (ob=0 ran interleaved into head-7 attention above)
                  with tc.tile_pool(name="cpsum", bufs=4,
                                    space="PSUM") as cps:
                      for ob in range(1, 4):
                          wo_t = cwp.tile([P, HG, 512], bf16, tag="cw")
                          nc.sync.dma_start(
                              wo_t, wo_r[:, :, ob * 512:(ob + 1) * 512])
                          for tch in range(NTC):
                              ps = cps.tile([P, 512], f32, tag="cps")
                              for h in range(HG):
                                  nc.tensor.matmul(
                                      ps, ot[h][:, tch * P:(tch + 1) * P],
                                      wo_t[:, h],
                                      start=(h == 0), stop=(h == HG - 1))
                              st = cstage.tile([P, 512], f32, tag="cst")
                              cp = (nc.vector.tensor_copy if tch % 2 == 0
                                    else nc.scalar.copy)
                              cp(out=st, in_=ps)
                              nc.sync.dma_start(
                                  out_d[tch * P:(tch + 1) * P,
                                        ob * 512:(ob + 1) * 512], st)

    nc.finalize()
    return nc


def _get_nc():
    global _CACHED
    if _CACHED is None:
        _CACHED = _build()
    return _CACHED


def _prep_inputs(x, W_qkv, W_out, vmode=None):
    """Host-side shard + layout prep. Returns per-core input maps."""
    import ml_dtypes
    bf16 = ml_dtypes.bfloat16
    f32 = np.float32
    x = np.asarray(x, dtype=f32)
    W_qkv = np.asarray(W_qkv, dtype=f32)
    W_out = np.asarray(W_out, dtype=f32)

    k_idx = np.arange(P)
    q_idx = np.arange(P)
    tri = (q_idx[None, :] >= k_idx[:, None]).astype(bf16)   # [k, q]
    ones = np.ones((P, 1), dtype=bf16)

    per_g = {}
    for g in range(2):
        sl = slice(g * HG * D, (g + 1) * HG * D)
        wq = W_qkv[0 * C:1 * C][sl]        # [1024, 2048]
        wk = W_qkv[1 * C:2 * C][sl]
        wv = W_qkv[2 * C:3 * C][sl]
        # [h, p, cs, m]: element = w[h*128+m, cs*128+p]
        def lay(w):
            return np.ascontiguousarray(
                w.reshape(HG, D, NCS, P).transpose(0, 3, 2, 1)).astype(bf16)
        wo_a = np.ascontiguousarray(W_out[:, sl].T).astype(bf16)  # [1024, 2048]
        per_g[g] = (lay(wq), lay(wk), lay(wv), wo_a)

    in_maps = []
    for core in range(8):
        b, g = divmod(core, 2)
        xt = np.ascontiguousarray(x[b].T).reshape(NCS, P, T).astype(bf16)
        wq_a, wk_a, wv_a, wo_a = per_g[g]
        im = {
            "xt": xt, "wq": wq_a, "wk": wk_a, "wv": wv_a, "wo": wo_a,
            "tri": tri, "ones": ones,
        }
        in_maps.append(im)
    return in_maps


def kernel(x, W_qkv, W_out, *, trace=False, trace_cores=None):
    from concourse.bass_utils import run_bass_kernel_spmd

    nc = _get_nc()
    in_maps = _prep_inputs(x, W_qkv, W_out)
    r = run_bass_kernel_spmd(
        nc, in_maps, core_ids=list(range(8)),
        trace=trace, trace_cores=trace_cores)

    out = np.empty((B, T, C), dtype=np.float32)
    for b in range(B):
        out[b] = r.results[2 * b]["out"] + r.results[2 * b + 1]["out"]
    if trace:
        kernel.last_results = r
    return out


# revision 30
# speedup vs baseline: 1.3609x; 1.3609x over previous
"""Causal self-attention Trainium2 kernel (8 NeuronCores, SPMD).

Problem (hardcoded): x [4, 2048, 2048] f32, W_qkv [6144, 2048], W_out [2048, 2048],
16 heads x 128 dim, causal softmax attention + output projection.

Sharding: core c = 2*b + g handles batch b (4) and head-group g (2 groups of 8
heads).  Each core computes its 8 heads' QKV projection, attention, and the
partial output projection against its slice of W_out columns; the host sums the
two partials per batch element.

v2 design: everything runs in bf16 on the PE (1 cycle/row, same rate as f32r,
but with no >=256 moving-width restriction, so causal tiles use exact widths).
The kernel is a single per-head pipeline: while attention of head h runs
(ACT exp / DVE softmax work), the QKV projection matmuls of head h+1 are
interleaved on the PE, so the scalar/vector work hides under the PE shadow.
Q^T/K^T/V stay in SBUF per head (no DRAM roundtrip).  The softmax denominator
is accumulated with bf16 DVE chain adds and reduced with a single ones-matmul
per (head, q-block); the reciprocal is broadcast with a tiny PE matmul.
"""

import math
from collections import deque

import numpy as np

B = 4
T = 2048
C = 2048
H = 16          # total heads
HG = 8          # heads per core (tensor-parallel group)
D = 128         # head dim
P = 128         # partitions
NCS = C // P    # 16 contraction subtiles
NTC = T // P    # 16 T chunks of 128
NTB = T // 512  # 4 T blocks of 512
SCALE = 1.0 / math.sqrt(D)

_CACHED = None
VMODE = "bf16"


def _build(phases="abc", repeat=1, vmode=None):
    import concourse.mybir as mybir
    from concourse import bacc
    from concourse.tile import TileContext

    f32 = mybir.dt.float32
    f32r = mybir.dt.float32r
    bf16 = mybir.dt.bfloat16
    EXP = mybir.ActivationFunctionType.Exp
    MULT = mybir.AluOpType.mult
    ADD = mybir.AluOpType.add

    nc = bacc.Bacc("TRN2", target_bir_lowering=False)

    xt_d = nc.dram_tensor("xt", [NCS, P, T], bf16, kind="ExternalInput")
    wq_d = nc.dram_tensor("wq", [HG, P, NCS, D], bf16, kind="ExternalInput")
    wk_d = nc.dram_tensor("wk", [HG, P, NCS, D], bf16, kind="ExternalInput")
    wv_d = nc.dram_tensor("wv", [HG, P, NCS, D], bf16, kind="ExternalInput")
    wo_d = nc.dram_tensor("wo", [HG * D, C], bf16, kind="ExternalInput")
    tri_d = nc.dram_tensor("tri", [P, P], bf16, kind="ExternalInput")
    ones_d = nc.dram_tensor("ones", [P, 1], bf16, kind="ExternalInput")
    out_d = nc.dram_tensor("out", [T, C], f32, kind="ExternalOutput")

    with TileContext(nc) as tc:
        with tc.tile_pool(name="persist", bufs=1) as persist:
            tri_t = persist.tile([P, P], bf16, tag="tri")
            nc.sync.dma_start(tri_t, tri_d[:])
            ones_t = persist.tile([P, 1], bf16, tag="ones")
            nc.sync.dma_start(ones_t, ones_d[:])

            for _rep in range(repeat):
                with tc.tile_pool(name="ot", bufs=1) as otp:
                  with tc.tile_pool(name="xt", bufs=1) as xtp, \
                     tc.tile_pool(name="wp", bufs=6) as wp, \
                     tc.tile_pool(name="qkv", bufs=2) as qkvsb, \
                     tc.tile_pool(name="pt", bufs=20) as ptp, \
                     tc.tile_pool(name="acc", bufs=2) as accp, \
                     tc.tile_pool(name="misc", bufs=4) as mscp, \
                     tc.tile_pool(name="qkvp", bufs=2, space="PSUM") as qkvp, \
                     tc.tile_pool(name="sp", bufs=4, space="PSUM") as spp, \
                     tc.tile_pool(name="pop", bufs=1, space="PSUM") as pop, \
                     tc.tile_pool(name="dnp", bufs=1, space="PSUM") as dnp:

                    # x^T resident in SBUF (bf16, 64 KiB/partition)
                    xt = []
                    for cs in range(NCS):
                        t_ = xtp.tile([P, T], bf16, tag=f"xt{cs}")
                        nc.sync.dma_start(t_, xt_d[cs])
                        xt.append(t_)

                    def load_w(h):
                        wq_t = wp.tile([P, NCS, D], bf16, tag="w")
                        nc.sync.dma_start(wq_t, wq_d[h])
                        wk_t = wp.tile([P, NCS, D], bf16, tag="w")
                        nc.sync.dma_start(wk_t, wk_d[h])
                        wv_t = wp.tile([P, NCS, D], bf16, tag="w")
                        nc.sync.dma_start(wv_t, wv_d[h])
                        return wq_t, wk_t, wv_t

                    def qkv_units(h, wts):
                        """12 closures; each issues one psum tile's worth of
                        QKV projection matmuls for head h plus its staging
                        copy."""
                        wq_t, wk_t, wv_t = wts
                        qt_t = qkvsb.tile([P, T], bf16, tag="qt")
                        kt_t = qkvsb.tile([P, T], bf16, tag="kt")
                        v_t = qkvsb.tile([P, T], bf16, tag="vt")
                        units = []
                        for w_t, dst in ((wq_t, qt_t), (wk_t, kt_t)):
                            for tb in range(NTB):
                                def u(w_t=w_t, dst=dst, tb=tb):
                                    ps = qkvp.tile([P, 512], f32, tag="qp")
                                    for cs in range(NCS):
                                        nc.tensor.matmul(
                                            ps, w_t[:, cs],
                                            xt[cs][:, tb * 512:(tb + 1) * 512],
                                            start=(cs == 0),
                                            stop=(cs == NCS - 1))
                                    nc.vector.tensor_copy(
                                        out=dst[:, tb * 512:(tb + 1) * 512],
                                        in_=ps)
                                units.append(u)
                        for g in range(4):
                            def u(g=g, wv_t=wv_t, v_t=v_t):
                                ps = qkvp.tile([P, 512], f32, tag="qp")
                                for j in range(4):
                                    tc_ = 4 * g + j
                                    for cs in range(NCS):
                                        nc.tensor.matmul(
                                            ps[:, j * P:(j + 1) * P],
                                            xt[cs][:, tc_ * P:(tc_ + 1) * P],
                                            wv_t[:, cs],
                                            start=(cs == 0),
                                            stop=(cs == NCS - 1))
                                nc.vector.tensor_copy(
                                    out=v_t[:, g * 512:(g + 1) * 512],
                                    in_=ps)
                            units.append(u)
                        return units, (qt_t, kt_t, v_t)

                    def att(h, qkv, next_unit):
                        qt_t, kt_t, v_t = qkv
                        ot_h = otp.tile([P, T], bf16, tag=f"ot{h}")
                        for jb in range(NTB):
                            nk = 4 * (jb + 1)
                            pts = []
                            acc = None
                            for g in range(jb + 1):
                                for j in range(4):
                                    ks = 4 * g + j
                                    m = ks - 4 * jb
                                    q0 = m * P if m > 0 else 0
                                    sp_t = spp.tile([P, 512], f32, tag="sp")
                                    nc.tensor.matmul(
                                        sp_t[:, q0:],
                                        kt_t[:, ks * P:(ks + 1) * P],
                                        qt_t[:, jb * 512 + q0:(jb + 1) * 512],
                                        start=True, stop=True)
                                    pt = ptp.tile([P, 512], bf16, tag="pt")
                                    nc.scalar.activation(
                                        pt[:, q0:], sp_t[:, q0:], EXP,
                                        scale=SCALE)
                                    if m >= 0:
                                        nc.vector.tensor_tensor(
                                            pt[:, q0:q0 + P],
                                            pt[:, q0:q0 + P], tri_t, MULT)
                                    pts.append((pt, q0))
                                    if ks == 0:
                                        acc = accp.tile([P, 512], bf16,
                                                        tag="acc")
                                        nc.vector.tensor_copy(out=acc, in_=pt)
                                    else:
                                        nc.vector.tensor_tensor(
                                            acc[:, q0:], acc[:, q0:],
                                            pt[:, q0:], ADD)
                                next_unit()
                            # denominator reduce + recip + broadcast issue
                            # before AV so DVE recip and the Pool broadcast
                            # overlap the AV matmuls on PE.
                            dn = dnp.tile([1, 512], f32, tag="dn")
                            nc.tensor.matmul(dn, ones_t, acc,
                                             start=True, stop=True)
                            rc = mscp.tile([1, 512], f32r, tag="rc")
                            with nc.allow_low_precision("softmax recip"):
                                nc.vector.reciprocal(rc, dn)
                            rb = mscp.tile([P, 512], f32r, tag="rb")
                            nc.gpsimd.partition_broadcast(rb, rc)
                            po = pop.tile([P, 512], f32, tag="po")
                            for ks in range(nk):
                                pt, q0 = pts[ks]
                                nc.tensor.matmul(
                                    po[:, q0:], v_t[:, ks * P:(ks + 1) * P],
                                    pt[:, q0:],
                                    start=(ks == 0), stop=(ks == nk - 1))
                            nc.vector.tensor_tensor(
                                ot_h[:, jb * 512:(jb + 1) * 512], po, rb,
                                MULT)
                        return ot_h

                    # head 0 QKV runs un-shadowed; weights for head 1
                    # prefetch during it.
                    wts0 = load_w(0)
                    units0, qkv0 = qkv_units(0, wts0)
                    wts_next = load_w(1)
                    for u in units0:
                        u()

                    ot = []
                    pending = deque()
                    qkv_cur = qkv0
                    for h in range(HG):
                        if h + 1 < HG:
                            units, qkv_next = qkv_units(h + 1, wts_next)
                            pending.extend(units)
                        if h + 2 < HG:
                            wts_next = load_w(h + 2)

                        def next_unit():
                            if pending:
                                pending.popleft()()

                        ot.append(att(h, qkv_cur, next_unit))
                        while pending:
                            pending.popleft()()
                        if h + 1 < HG:
                            qkv_cur = qkv_next

                  # ---------------- output projection ----------------
                  wo_r = wo_d.rearrange("(h p) o -> p h o", p=P)
                  with tc.tile_pool(name="cw", bufs=2) as cwp, \
                       tc.tile_pool(name="cstage", bufs=4) as cstage, \
                       tc.tile_pool(name="cpsum", bufs=4,
                                    space="PSUM") as cps:
                      for ob in range(4):
                          wo_t = cwp.tile([P, HG, 512], bf16, tag="cw")
                          nc.sync.dma_start(
                              wo_t, wo_r[:, :, ob * 512:(ob + 1) * 512])
                          for tch in range(NTC):
                              ps = cps.tile([P, 512], f32, tag="cps")
                              for h in range(HG):
                                  nc.tensor.matmul(
                                      ps, ot[h][:, tch * P:(tch + 1) * P],
                                      wo_t[:, h],
                                      start=(h == 0), stop=(h == HG - 1))
                              st = cstage.tile([P, 512], f32, tag="cst")
                              nc.vector.tensor_copy(out=st, in_=ps)
                              nc.sync.dma_start(
                                  out_d[tch * P:(tch + 1) * P,
                                        ob * 512:(ob + 1) * 512], st)

    nc.finalize()
    return nc


def _get_nc():
    global _CACHED
    if _CACHED is None:
        _CACHED = _build()
    return _CACHED


def _prep_inputs(x, W_qkv, W_out, vmode=None):
    """Host-side shard + layout prep. Returns per-core input maps."""
    import ml_dtypes
    bf16 = ml_dtypes.bfloat16
    f32 = np.float32
    x = np.asarray(x, dtype=f32)
    W_qkv = np.asarray(W_qkv, dtype=f32)
    W_out = np.asarray(W_out, dtype=f32)

    k_idx = np.arange(P)
    q_idx = np.arange(P)
    tri = (q_idx[None, :] >= k_idx[:, None]).astype(bf16)   # [k, q]
    ones = np.ones((P, 1), dtype=bf16)

    per_g = {}
    for g in range(2):
        sl = slice(g * HG * D, (g + 1) * HG * D)
        wq = W_qkv[0 * C:1 * C][sl]        # [1024, 2048]
        wk = W_qkv[1 * C:2 * C][sl]
        wv = W_qkv[2 * C:3 * C][sl]
        # [h, p, cs, m]: element = w[h*128+m, cs*128+p]
        def lay(w):
            return np.ascontiguousarray(
                w.reshape(HG, D, NCS, P).transpose(0, 3, 2, 1)).astype(bf16)
        wo_a = np.ascontiguousarray(W_out[:, sl].T).astype(bf16)  # [1024, 2048]
        per_g[g] = (lay(wq), lay(wk), lay(wv), wo_a)

    in_maps = []
    for core in range(8):
        b, g = divmod(core, 2)
        xt = np.ascontiguousarray(x[b].T).reshape(NCS, P, T).astype(bf16)
        wq_a, wk_a, wv_a, wo_a = per_g[g]
        im = {
            "xt": xt, "wq": wq_a, "wk": wk_a, "wv": wv_a, "wo": wo_a,
            "tri": tri, "ones": ones,
        }
        in_maps.append(im)
    return in_maps


def kernel(x, W_qkv, W_out, *, trace=False, trace_cores=None):
    from concourse.bass_utils import run_bass_kernel_spmd

    nc = _get_nc()
    in_maps = _prep_inputs(x, W_qkv, W_out)
    r = run_bass_kernel_spmd(
        nc, in_maps, core_ids=list(range(8)),
        trace=trace, trace_cores=trace_cores)

    out = np.empty((B, T, C), dtype=np.float32)
    for b in range(B):
        out[b] = r.results[2 * b]["out"] + r.results[2 * b + 1]["out"]
    if trace:
        kernel.last_results = r
    return out
